# revision 4
# baseline (speedup 1.0000x reference)
"""SwiGLU MLP (CUTLASS-style fused gate/up) on 8 TRN2 NeuronCores.

Reference computation (all f32):
    x12 = x @ w12.T + b12          # [B,S,2m], w12: [2m,k]
    x1, x2 = split(x12, 2)         # gate, up
    x4 = silu(x1) * x2             # [B,S,m]
    out = x4 @ w3.T + b3           # [B,S,m]

Sharding: pure data-parallel over the 8192 tokens (1024 tokens/core),
weights replicated. No collectives. Compute in bf16 with f32 PSUM
accumulation; biases and output in f32.

Per-core layout (everything packed on host into SBUF-friendly tiles):
  xp   [128, KT*T]   bf16   xp[p, kt*T + t]       = x_c[t, kt*128+p]
  w12p [2*MT,128,KT*128] bf16  w12p[jt,p,kt*128+jj] = w12[jt*128+jj, kt*128+p]
  w3p  [MT, 128,MT*128]  bf16  w3p[nt,p,mt*128+nn]  = w3[nt*128+nn, mt*128+p]
  b12p [128, 2*MT]  f32    b12p[p, jt]           = b12[jt*128+p]
  b3p  [128, MT]    f32    b3p[p, nt]            = b3[nt*128+p]
  out  [MT, 128, T] f32    out[nt, p, t]         = y_c[t, nt*128+p]

Matmul convention: psum[M,N] = lhsT[K,M].T @ rhs[K,N], K on partitions.
Phase 1: lhsT = w12p block slices, rhs = xp slices -> psum [j, t_chunk].
Phase 2: lhsT = w3p block slices, rhs = x4 (SBUF-resident) -> psum [n, t].
"""

import os
import sys
import types

sys.path.insert(0, "/opt/trn_rl_repo")

import numpy as np
import ml_dtypes

BF16 = ml_dtypes.bfloat16

B, S = 4, 2048
K = 4096          # input dim
M = 4096          # hidden / output dim
NCORES = 8
TOK = B * S       # 8192 tokens
T = TOK // NCORES # 1024 tokens per core
P = 128
KT = K // P       # 32 k-tiles
MT = M // P       # 32 m/n-tiles
JT = 2 * MT       # 64 gate+up tiles
NF = 512          # matmul moving free dim (one PSUM bank)
TC = T // NF      # 2 token chunks per core

_COMPILED = {}
LAST_EXEC_NS = None


def _register_ntff_hook():
    """run_bass_kernel_spmd(trace=True) under axon needs
    antenv.axon_hooks, which this image doesn't ship. Register an
    equivalent built from trn_boot's ctypes helper so traced runs work."""
    if "antenv.axon_hooks" in sys.modules:
        return
    try:
        from trn_agent_boot.trn_boot import _ntff_profile_via_ctypes
        hook = _ntff_profile_via_ctypes("/opt/axon/libaxon_pjrt.so")
    except Exception:
        hook = None
    mod = types.ModuleType("antenv.axon_hooks")
    _h = [hook]
    mod.set_axon_ntff_profile_hook = lambda h: _h.__setitem__(0, h)
    mod.get_axon_ntff_profile_hook = lambda: _h[0]
    sys.modules["antenv.axon_hooks"] = mod


def _build():
    """Build + finalize the per-core Bass module (shared by all 8 cores)."""
    import concourse.mybir as mybir
    import concourse.tile as tile
    from concourse import bacc

    f32 = mybir.dt.float32
    bf16 = mybir.dt.bfloat16

    nc = bacc.Bacc("TRN2", target_bir_lowering=False, debug=False,
                   num_devices=NCORES)

    xp_d = nc.dram_tensor("xp", [P, KT * T], bf16, kind="ExternalInput").ap()
    w12_d = nc.dram_tensor("w12p", [JT, P, KT * P], bf16, kind="ExternalInput").ap()
    w3_d = nc.dram_tensor("w3p", [MT, P, MT * P], bf16, kind="ExternalInput").ap()
    b12_d = nc.dram_tensor("b12p", [P, JT], f32, kind="ExternalInput").ap()
    b3_d = nc.dram_tensor("b3p", [P, MT], f32, kind="ExternalInput").ap()
    out_d = nc.dram_tensor("out", [MT, P, T], f32, kind="ExternalOutput").ap()

    with tile.TileContext(nc) as tc:
        with (
            tc.tile_pool(name="resident", bufs=1) as res_pool,
            tc.tile_pool(name="weights", bufs=3) as w_pool,
            tc.tile_pool(name="w3pool", bufs=2) as w3_pool,
            tc.tile_pool(name="evict", bufs=3) as act_pool,
            tc.tile_pool(name="psum", bufs=2, space="PSUM") as ps_pool,
        ):
            xs = res_pool.tile([P, KT * T], bf16, tag="xs")
            x4 = res_pool.tile([P, MT * T], bf16, tag="x4")

            def load_w12_block(idx):
                t = w_pool.tile([P, KT * P], bf16, tag="w")
                nc.sync.dma_start(t[:], w12_d[idx])
                return t

            # Startup critical path is dma_start ISSUE serialization
            # (~0.6us each on the sync sequencer), so: few large DMAs,
            # in first-consumption order — i=0 weights, then xs halves.
            wg0 = load_w12_block(0)
            wu0 = load_w12_block(MT)
            H = KT * T // 2
            nc.sync.dma_start(xs[:, :H], xp_d[:, :H])
            nc.sync.dma_start(xs[:, H:], xp_d[:, H:])
            b12s = res_pool.tile([P, JT], f32, tag="b12")
            nc.sync.dma_start(b12s[:], b12_d[:])
            b3s = res_pool.tile([P, MT], f32, tag="b3")
            nc.sync.dma_start(b3s[:], b3_d[:])

            # ---- Phase 1: x4[:, i*T + t] = silu(gate) * (up + b_up) ----
            for i in range(MT):
                if i == 0:
                    wg, wu = wg0, wu0
                else:
                    wg = load_w12_block(i)
                    wu = load_w12_block(MT + i)
                for tcn in range(TC):
                    rhs = xs  # [P, KT*T]; slice per kt below
                    psg = ps_pool.tile([P, NF], f32, tag="psg")
                    for kt in range(KT):
                        nc.tensor.matmul(
                            psg[:],
                            wg[:, kt * P:(kt + 1) * P],
                            rhs[:, kt * T + tcn * NF: kt * T + tcn * NF + NF],
                            start=(kt == 0), stop=(kt == KT - 1),
                        )
                    psu = ps_pool.tile([P, NF], f32, tag="psu")
                    for kt in range(KT):
                        nc.tensor.matmul(
                            psu[:],
                            wu[:, kt * P:(kt + 1) * P],
                            rhs[:, kt * T + tcn * NF: kt * T + tcn * NF + NF],
                            start=(kt == 0), stop=(kt == KT - 1),
                        )
                    # silu(gate + b_gate) on ScalarE (f32)
                    actg = act_pool.tile([P, NF], f32, tag="actg")
                    nc.scalar.activation(
                        actg[:], psg[:],
                        mybir.ActivationFunctionType.Silu,
                        bias=b12s[:, i:i + 1],
                    )
                    # x4 = (up + b_up) * silu_result on VectorE, cast bf16
                    nc.vector.scalar_tensor_tensor(
                        x4[:, i * T + tcn * NF: i * T + tcn * NF + NF],
                        psu[:],
                        b12s[:, MT + i:MT + i + 1],
                        actg[:],
                        mybir.AluOpType.add,
                        mybir.AluOpType.mult,
                    )

            # ---- Phase 2: out[nt] = w3p[nt].T-blocks @ x4 + b3 ----
            for nt in range(MT):
                w3t = w3_pool.tile([P, MT * P], bf16, tag="w3")
                nc.sync.dma_start(w3t[:], w3_d[nt])
                for tcn in range(TC):
                    pso = ps_pool.tile([P, NF], f32, tag="pso")
                    for mt in range(MT):
                        nc.tensor.matmul(
                            pso[:],
                            w3t[:, mt * P:(mt + 1) * P],
                            x4[:, mt * T + tcn * NF: mt * T + tcn * NF + NF],
                            start=(mt == 0), stop=(mt == MT - 1),
                        )
                    outsb = act_pool.tile([P, NF], f32, tag="outsb")
                    nc.scalar.add(outsb[:], pso[:], b3s[:, nt:nt + 1])
                    nc.sync.dma_start(
                        out_d[nt, :, tcn * NF:(tcn + 1) * NF], outsb[:]
                    )

    nc.finalize()
    return nc


def _get_compiled():
    if "nc" not in _COMPILED:
        _register_ntff_hook()
        _COMPILED["nc"] = _build()
    return _COMPILED["nc"]


def _pack_inputs(x, w12, b12, w3, b3):
    """Host-side packing into the per-core DRAM layouts (see header)."""
    xf = np.ascontiguousarray(x, dtype=np.float32).reshape(TOK, K)
    w12p = (
        w12.astype(BF16)
        .reshape(JT, P, KT, P)        # [jt, jj, kt, p]
        .transpose(0, 3, 2, 1)        # [jt, p, kt, jj]
        .reshape(JT, P, KT * P)
    )
    w12p = np.ascontiguousarray(w12p)
    w3p = (
        w3.astype(BF16)
        .reshape(MT, P, MT, P)        # [nt, nn, mt, p]
        .transpose(0, 3, 2, 1)        # [nt, p, mt, nn]
        .reshape(MT, P, MT * P)
    )
    w3p = np.ascontiguousarray(w3p)
    b12p = np.ascontiguousarray(
        b12.astype(np.float32).reshape(JT, P).T)
    b3p = np.ascontiguousarray(
        b3.astype(np.float32).reshape(MT, P).T)

    in_maps = []
    for c in range(NCORES):
        xc = xf[c * T:(c + 1) * T]    # [T, K]
        xp = (
            xc.astype(BF16).T          # [K, T]
            .reshape(KT, P, T)
            .transpose(1, 0, 2)        # [p, kt, t]
            .reshape(P, KT * T)
        )
        in_maps.append({
            "xp": np.ascontiguousarray(xp),
            "w12p": w12p,
            "w3p": w3p,
            "b12p": b12p,
            "b3p": b3p,
        })
    return in_maps


def kernel(x, w12, b12, w3, b3):
    global LAST_EXEC_NS
    from concourse.bass_utils import run_bass_kernel_spmd

    nc = _get_compiled()
    in_maps = _pack_inputs(x, w12, b12, w3, b3)
    trace = os.environ.get("KERNEL_TRACE", "0") == "1"
    res = run_bass_kernel_spmd(
        nc, in_maps, core_ids=list(range(NCORES)), trace=trace
    )
    LAST_EXEC_NS = res.exec_time_ns

    outs = []
    for c in range(NCORES):
        o = res.results[c]["out"]               # [MT, P, T]
        outs.append(np.transpose(o, (2, 0, 1)).reshape(T, M))
    y = np.concatenate(outs, axis=0).reshape(B, S, M)
    return np.ascontiguousarray(y, dtype=np.float32)


# revision 6
# speedup vs baseline: 1.0024x; 1.0024x over previous
"""SwiGLU MLP (CUTLASS-style fused gate/up) on 8 TRN2 NeuronCores.

Reference computation (all f32):
    x12 = x @ w12.T + b12          # [B,S,2m], w12: [2m,k]
    x1, x2 = split(x12, 2)         # gate, up
    x4 = silu(x1) * x2             # [B,S,m]
    out = x4 @ w3.T + b3           # [B,S,m]

Sharding: pure data-parallel over the 8192 tokens (1024 tokens/core),
weights replicated. No collectives. Compute in bf16 with f32 PSUM
accumulation; biases and output in f32.

Per-core layout (everything packed on host into SBUF-friendly tiles):
  xp   [128, KT*T]   bf16   xp[p, kt*T + t]       = x_c[t, kt*128+p]
  w12p [2*MT,128,KT*128] bf16  w12p[jt,p,kt*128+jj] = w12[jt*128+jj, kt*128+p]
  w3p  [MT, 128,MT*128]  bf16  w3p[nt,p,mt*128+nn]  = w3[nt*128+nn, mt*128+p]
  b12p [128, 2*MT]  f32    b12p[p, jt]           = b12[jt*128+p]
  b3p  [128, MT]    f32    b3p[p, nt]            = b3[nt*128+p]
  out  [MT, 128, T] f32    out[nt, p, t]         = y_c[t, nt*128+p]

Matmul convention: psum[M,N] = lhsT[K,M].T @ rhs[K,N], K on partitions.
Phase 1: lhsT = w12p block slices, rhs = xp slices -> psum [j, t_chunk].
Phase 2: lhsT = w3p block slices, rhs = x4 (SBUF-resident) -> psum [n, t].
"""

import os
import sys
import types

sys.path.insert(0, "/opt/trn_rl_repo")

import numpy as np
import ml_dtypes

BF16 = ml_dtypes.bfloat16

B, S = 4, 2048
K = 4096          # input dim
M = 4096          # hidden / output dim
NCORES = 8
TOK = B * S       # 8192 tokens
T = TOK // NCORES # 1024 tokens per core
P = 128
KT = K // P       # 32 k-tiles
MT = M // P       # 32 m/n-tiles
JT = 2 * MT       # 64 gate+up tiles
NF = 512          # matmul moving free dim (one PSUM bank)
TC = T // NF      # 2 token chunks per core

_COMPILED = {}
LAST_EXEC_NS = None


def _register_ntff_hook():
    """run_bass_kernel_spmd(trace=True) under axon needs
    antenv.axon_hooks, which this image doesn't ship. Register an
    equivalent built from trn_boot's ctypes helper so traced runs work."""
    if "antenv.axon_hooks" in sys.modules:
        return
    try:
        from trn_agent_boot.trn_boot import _ntff_profile_via_ctypes
        hook = _ntff_profile_via_ctypes("/opt/axon/libaxon_pjrt.so")
    except Exception:
        hook = None
    mod = types.ModuleType("antenv.axon_hooks")
    _h = [hook]
    mod.set_axon_ntff_profile_hook = lambda h: _h.__setitem__(0, h)
    mod.get_axon_ntff_profile_hook = lambda: _h[0]
    sys.modules["antenv.axon_hooks"] = mod


def _build():
    """Build + finalize the per-core Bass module (shared by all 8 cores)."""
    import concourse.mybir as mybir
    import concourse.tile as tile
    from concourse import bacc

    f32 = mybir.dt.float32
    bf16 = mybir.dt.bfloat16

    nc = bacc.Bacc("TRN2", target_bir_lowering=False, debug=False,
                   num_devices=NCORES)

    xp_d = nc.dram_tensor("xp", [P, KT * T], bf16, kind="ExternalInput").ap()
    w12_d = nc.dram_tensor("w12p", [JT, P, KT * P], bf16, kind="ExternalInput").ap()
    w3_d = nc.dram_tensor("w3p", [MT, P, MT * P], bf16, kind="ExternalInput").ap()
    b12_d = nc.dram_tensor("b12p", [P, JT], f32, kind="ExternalInput").ap()
    b3_d = nc.dram_tensor("b3p", [P, MT], f32, kind="ExternalInput").ap()
    out_d = nc.dram_tensor("out", [MT, P, T], f32, kind="ExternalOutput").ap()

    with tile.TileContext(nc) as tc:
        with (
            tc.tile_pool(name="resident", bufs=1) as res_pool,
            tc.tile_pool(name="weights", bufs=3) as w_pool,
            tc.tile_pool(name="w3pool", bufs=2) as w3_pool,
            tc.tile_pool(name="evict", bufs=3) as act_pool,
            tc.tile_pool(name="psum", bufs=2, space="PSUM") as ps_pool,
        ):
            xs = res_pool.tile([P, KT * T], bf16, tag="xs")
            x4 = res_pool.tile([P, MT * T], bf16, tag="x4")
            b12s = res_pool.tile([P, JT], f32, tag="b12")
            nc.sync.dma_start(b12s[:], b12_d[:])
            b3s = res_pool.tile([P, MT], f32, tag="b3")
            nc.sync.dma_start(b3s[:], b3_d[:])

            def load_w12_block(idx):
                t = w_pool.tile([P, KT * P], bf16, tag="w")
                nc.sync.dma_start(t[:], w12_d[idx])
                return t

            # First gate/up weight pair ahead of the bulk xs load so the
            # PE can start as soon as block 0 + xs chunk 0 land. Chunked
            # so matmul kt only waits on the quarter-block holding kt.
            def load_w12_block_chunked(idx, nchunks=4):
                t = w_pool.tile([P, KT * P], bf16, tag="w")
                cw = KT * P // nchunks
                for ci in range(nchunks):
                    nc.sync.dma_start(
                        t[:, ci * cw:(ci + 1) * cw],
                        w12_d[idx, :, ci * cw:(ci + 1) * cw],
                    )
                return t

            wg0 = load_w12_block_chunked(0)
            wu0 = load_w12_block_chunked(MT)
            # xs chunked per k-tile: matmul kt depends only on chunk kt,
            # letting the PE ramp while later chunks stream in.
            for kt in range(KT):
                nc.sync.dma_start(
                    xs[:, kt * T:(kt + 1) * T], xp_d[:, kt * T:(kt + 1) * T]
                )

            # ---- Phase 1: x4[:, i*T + t] = silu(gate) * (up + b_up) ----
            for i in range(MT):
                if i == 0:
                    wg, wu = wg0, wu0
                else:
                    wg = load_w12_block(i)
                    wu = load_w12_block(MT + i)
                for tcn in range(TC):
                    rhs = xs  # [P, KT*T]; slice per kt below
                    psg = ps_pool.tile([P, NF], f32, tag="psg")
                    for kt in range(KT):
                        nc.tensor.matmul(
                            psg[:],
                            wg[:, kt * P:(kt + 1) * P],
                            rhs[:, kt * T + tcn * NF: kt * T + tcn * NF + NF],
                            start=(kt == 0), stop=(kt == KT - 1),
                        )
                    psu = ps_pool.tile([P, NF], f32, tag="psu")
                    for kt in range(KT):
                        nc.tensor.matmul(
                            psu[:],
                            wu[:, kt * P:(kt + 1) * P],
                            rhs[:, kt * T + tcn * NF: kt * T + tcn * NF + NF],
                            start=(kt == 0), stop=(kt == KT - 1),
                        )
                    # silu(gate + b_gate) on ScalarE (f32)
                    actg = act_pool.tile([P, NF], f32, tag="actg")
                    nc.scalar.activation(
                        actg[:], psg[:],
                        mybir.ActivationFunctionType.Silu,
                        bias=b12s[:, i:i + 1],
                    )
                    # x4 = (up + b_up) * silu_result on VectorE, cast bf16
                    nc.vector.scalar_tensor_tensor(
                        x4[:, i * T + tcn * NF: i * T + tcn * NF + NF],
                        psu[:],
                        b12s[:, MT + i:MT + i + 1],
                        actg[:],
                        mybir.AluOpType.add,
                        mybir.AluOpType.mult,
                    )

            # ---- Phase 2: out[nt] = w3p[nt].T-blocks @ x4 + b3 ----
            for nt in range(MT):
                w3t = w3_pool.tile([P, MT * P], bf16, tag="w3")
                nc.sync.dma_start(w3t[:], w3_d[nt])
                for tcn in range(TC):
                    pso = ps_pool.tile([P, NF], f32, tag="pso")
                    for mt in range(MT):
                        nc.tensor.matmul(
                            pso[:],
                            w3t[:, mt * P:(mt + 1) * P],
                            x4[:, mt * T + tcn * NF: mt * T + tcn * NF + NF],
                            start=(mt == 0), stop=(mt == MT - 1),
                        )
                    outsb = act_pool.tile([P, NF], f32, tag="outsb")
                    nc.scalar.add(outsb[:], pso[:], b3s[:, nt:nt + 1])
                    nc.sync.dma_start(
                        out_d[nt, :, tcn * NF:(tcn + 1) * NF], outsb[:]
                    )

    nc.finalize()
    return nc


def _get_compiled():
    if "nc" not in _COMPILED:
        _register_ntff_hook()
        _COMPILED["nc"] = _build()
    return _COMPILED["nc"]


def _pack_inputs(x, w12, b12, w3, b3):
    """Host-side packing into the per-core DRAM layouts (see header)."""
    x, w12, b12, w3, b3 = (np.asarray(a) for a in (x, w12, b12, w3, b3))
    xf = np.ascontiguousarray(x, dtype=np.float32).reshape(TOK, K)
    w12p = (
        w12.astype(BF16)
        .reshape(JT, P, KT, P)        # [jt, jj, kt, p]
        .transpose(0, 3, 2, 1)        # [jt, p, kt, jj]
        .reshape(JT, P, KT * P)
    )
    w12p = np.ascontiguousarray(w12p)
    w3p = (
        w3.astype(BF16)
        .reshape(MT, P, MT, P)        # [nt, nn, mt, p]
        .transpose(0, 3, 2, 1)        # [nt, p, mt, nn]
        .reshape(MT, P, MT * P)
    )
    w3p = np.ascontiguousarray(w3p)
    b12p = np.ascontiguousarray(
        b12.astype(np.float32).reshape(JT, P).T)
    b3p = np.ascontiguousarray(
        b3.astype(np.float32).reshape(MT, P).T)

    in_maps = []
    for c in range(NCORES):
        xc = xf[c * T:(c + 1) * T]    # [T, K]
        xp = (
            xc.astype(BF16).T          # [K, T]
            .reshape(KT, P, T)
            .transpose(1, 0, 2)        # [p, kt, t]
            .reshape(P, KT * T)
        )
        in_maps.append({
            "xp": np.ascontiguousarray(xp),
            "w12p": w12p,
            "w3p": w3p,
            "b12p": b12p,
            "b3p": b3p,
        })
    return in_maps


def kernel(x, w12, b12, w3, b3):
    global LAST_EXEC_NS
    from concourse.bass_utils import run_bass_kernel_spmd

    nc = _get_compiled()
    in_maps = _pack_inputs(x, w12, b12, w3, b3)
    trace = os.environ.get("KERNEL_TRACE", "0") == "1"
    res = run_bass_kernel_spmd(
        nc, in_maps, core_ids=list(range(NCORES)), trace=trace
    )
    LAST_EXEC_NS = res.exec_time_ns

    outs = []
    for c in range(NCORES):
        o = res.results[c]["out"]               # [MT, P, T]
        outs.append(np.transpose(o, (2, 0, 1)).reshape(T, M))
    y = np.concatenate(outs, axis=0).reshape(B, S, M)
    return np.ascontiguousarray(y, dtype=np.float32)


# revision 11
# speedup vs baseline: 1.0032x; 1.0007x over previous
"""SwiGLU MLP (CUTLASS-style fused gate/up) on 8 TRN2 NeuronCores.

Reference computation (all f32):
    x12 = x @ w12.T + b12          # [B,S,2m], w12: [2m,k]
    x1, x2 = split(x12, 2)         # gate, up
    x4 = silu(x1) * x2             # [B,S,m]
    out = x4 @ w3.T + b3           # [B,S,m]

Sharding: pure data-parallel over the 8192 tokens (1024 tokens/core),
weights replicated. No collectives. Compute in bf16 with f32 PSUM
accumulation; biases and output in f32.

Per-core layout (everything packed on host into SBUF-friendly tiles):
  xp   [128, KT*T]   bf16   xp[p, kt*T + t]       = x_c[t, kt*128+p]
  w12p [2*MT,128,KT*128] bf16  w12p[jt,p,kt*128+jj] = w12[jt*128+jj, kt*128+p]
  w3p  [MT, 128,MT*128]  bf16  w3p[nt,p,mt*128+nn]  = w3[nt*128+nn, mt*128+p]
  b12p [128, 2*MT]  f32    b12p[p, jt]           = b12[jt*128+p]
  b3p  [128, MT]    f32    b3p[p, nt]            = b3[nt*128+p]
  out  [MT, 128, T] f32    out[nt, p, t]         = y_c[t, nt*128+p]

Matmul convention: psum[M,N] = lhsT[K,M].T @ rhs[K,N], K on partitions.
Phase 1: lhsT = w12p block slices, rhs = xp slices -> psum [j, t_chunk].
Phase 2: lhsT = w3p block slices, rhs = x4 (SBUF-resident) -> psum [n, t].
"""

import os
import sys
import types

sys.path.insert(0, "/opt/trn_rl_repo")

import numpy as np
import ml_dtypes

BF16 = ml_dtypes.bfloat16

B, S = 4, 2048
K = 4096          # input dim
M = 4096          # hidden / output dim
NCORES = 8
TOK = B * S       # 8192 tokens
T = TOK // NCORES # 1024 tokens per core
P = 128
KT = K // P       # 32 k-tiles
MT = M // P       # 32 m/n-tiles
JT = 2 * MT       # 64 gate+up tiles
NF = 512          # matmul moving free dim (one PSUM bank)
TC = T // NF      # 2 token chunks per core

_COMPILED = {}
LAST_EXEC_NS = None


def _register_ntff_hook():
    """run_bass_kernel_spmd(trace=True) under axon needs
    antenv.axon_hooks, which this image doesn't ship. Register an
    equivalent built from trn_boot's ctypes helper so traced runs work."""
    if "antenv.axon_hooks" in sys.modules:
        return
    try:
        from trn_agent_boot.trn_boot import _ntff_profile_via_ctypes
        hook = _ntff_profile_via_ctypes("/opt/axon/libaxon_pjrt.so")
    except Exception:
        hook = None
    mod = types.ModuleType("antenv.axon_hooks")
    _h = [hook]
    mod.set_axon_ntff_profile_hook = lambda h: _h.__setitem__(0, h)
    mod.get_axon_ntff_profile_hook = lambda: _h[0]
    sys.modules["antenv.axon_hooks"] = mod


def _build():
    """Build + finalize the per-core Bass module (shared by all 8 cores)."""
    import concourse.mybir as mybir
    import concourse.tile as tile
    from concourse import bacc

    f32 = mybir.dt.float32
    bf16 = mybir.dt.bfloat16

    nc = bacc.Bacc("TRN2", target_bir_lowering=False, debug=False,
                   num_devices=NCORES)

    xp_d = nc.dram_tensor("xp", [P, KT * T], bf16, kind="ExternalInput").ap()
    w12_d = nc.dram_tensor("w12p", [JT, P, KT * P], bf16, kind="ExternalInput").ap()
    w3_d = nc.dram_tensor("w3p", [MT, P, MT * P], bf16, kind="ExternalInput").ap()
    b12_d = nc.dram_tensor("b12p", [P, JT], f32, kind="ExternalInput").ap()
    b3_d = nc.dram_tensor("b3p", [P, MT], f32, kind="ExternalInput").ap()
    out_d = nc.dram_tensor("out", [MT, P, T], f32, kind="ExternalOutput").ap()

    with tile.TileContext(nc) as tc:
        with (
            tc.tile_pool(name="resident", bufs=1) as res_pool,
            tc.tile_pool(name="weights", bufs=3) as w_pool,
            tc.tile_pool(name="w3pool", bufs=2) as w3_pool,
            tc.tile_pool(name="evict", bufs=3) as act_pool,
            tc.tile_pool(name="psum", bufs=2, space="PSUM") as ps_pool,
        ):
            xs = res_pool.tile([P, KT * T], bf16, tag="xs")
            x4 = res_pool.tile([P, MT * T], bf16, tag="x4")

            b12s = res_pool.tile([P, JT], f32, tag="b12")
            nc.sync.dma_start(b12s[:], b12_d[:])
            b3s = res_pool.tile([P, MT], f32, tag="b3")
            nc.sync.dma_start(b3s[:], b3_d[:])

            def load_w12_block(idx):
                t = w_pool.tile([P, KT * P], bf16, tag="w")
                nc.sync.dma_start(t[:], w12_d[idx])
                return t

            # First gate/up weight pair ahead of the bulk xs load so the
            # PE can start as soon as block 0 + xs chunk 0 land. Chunked
            # so matmul kt only waits on the quarter-block holding kt.
            def load_w12_block_chunked(idx, nchunks=4):
                t = w_pool.tile([P, KT * P], bf16, tag="w")
                cw = KT * P // nchunks
                for ci in range(nchunks):
                    nc.sync.dma_start(
                        t[:, ci * cw:(ci + 1) * cw],
                        w12_d[idx, :, ci * cw:(ci + 1) * cw],
                    )
                return t

            wg0 = load_w12_block_chunked(0)
            wu0 = load_w12_block_chunked(MT)
            # xs chunked per k-tile: matmul kt depends only on chunk kt,
            # letting the PE ramp while later chunks stream in.
            for kt in range(KT):
                nc.sync.dma_start(
                    xs[:, kt * T:(kt + 1) * T], xp_d[:, kt * T:(kt + 1) * T]
                )

            # ---- Phase 1: x4[:, i*T + t] = silu(gate) * (up + b_up) ----
            # NOTE: gate/up matmul groups must stay sequential — alternating
            # PSUM banks per MM triggers HAM oscillation (~20% slower).
            for i in range(MT):
                if i == 0:
                    wg, wu = wg0, wu0
                else:
                    wg = load_w12_block(i)
                    wu = load_w12_block(MT + i)
                for tcn in range(TC):
                    psg = ps_pool.tile([P, NF], f32, tag="psg")
                    for kt in range(KT):
                        nc.tensor.matmul(
                            psg[:],
                            wg[:, kt * P:(kt + 1) * P],
                            xs[:, kt * T + tcn * NF: kt * T + tcn * NF + NF],
                            start=(kt == 0), stop=(kt == KT - 1),
                        )
                    psu = ps_pool.tile([P, NF], f32, tag="psu")
                    for kt in range(KT):
                        nc.tensor.matmul(
                            psu[:],
                            wu[:, kt * P:(kt + 1) * P],
                            xs[:, kt * T + tcn * NF: kt * T + tcn * NF + NF],
                            start=(kt == 0), stop=(kt == KT - 1),
                        )
                    # silu(gate + b_gate) on ScalarE (f32)
                    actg = act_pool.tile([P, NF], f32, tag="actg")
                    nc.scalar.activation(
                        actg[:], psg[:],
                        mybir.ActivationFunctionType.Silu,
                        bias=b12s[:, i:i + 1],
                    )
                    # x4 = (up + b_up) * silu_result on VectorE, cast bf16
                    nc.vector.scalar_tensor_tensor(
                        x4[:, i * T + tcn * NF: i * T + tcn * NF + NF],
                        psu[:],
                        b12s[:, MT + i:MT + i + 1],
                        actg[:],
                        mybir.AluOpType.add,
                        mybir.AluOpType.mult,
                    )

            # ---- Phase 2: out[nt] = w3p[nt].T-blocks @ x4 + b3 ----
            for nt in range(MT):
                w3t = w3_pool.tile([P, MT * P], bf16, tag="w3")
                nc.sync.dma_start(w3t[:], w3_d[nt])
                for tcn in range(TC):
                    pso = ps_pool.tile([P, NF], f32, tag="pso")
                    for mt in range(MT):
                        nc.tensor.matmul(
                            pso[:],
                            w3t[:, mt * P:(mt + 1) * P],
                            x4[:, mt * T + tcn * NF: mt * T + tcn * NF + NF],
                            start=(mt == 0), stop=(mt == MT - 1),
                        )
                    outsb = act_pool.tile([P, NF], f32, tag="outsb")
                    nc.scalar.add(outsb[:], pso[:], b3s[:, nt:nt + 1])
                    nc.sync.dma_start(
                        out_d[nt, :, tcn * NF:(tcn + 1) * NF], outsb[:]
                    )

    nc.finalize()
    return nc


def _get_compiled():
    if "nc" not in _COMPILED:
        _register_ntff_hook()
        _COMPILED["nc"] = _build()
    return _COMPILED["nc"]


def _pack_inputs(x, w12, b12, w3, b3):
    """Host-side packing into the per-core DRAM layouts (see header)."""
    x, w12, b12, w3, b3 = (np.asarray(a) for a in (x, w12, b12, w3, b3))
    xf = np.ascontiguousarray(x, dtype=np.float32).reshape(TOK, K)
    w12p = (
        w12.astype(BF16)
        .reshape(JT, P, KT, P)        # [jt, jj, kt, p]
        .transpose(0, 3, 2, 1)        # [jt, p, kt, jj]
        .reshape(JT, P, KT * P)
    )
    w12p = np.ascontiguousarray(w12p)
    w3p = (
        w3.astype(BF16)
        .reshape(MT, P, MT, P)        # [nt, nn, mt, p]
        .transpose(0, 3, 2, 1)        # [nt, p, mt, nn]
        .reshape(MT, P, MT * P)
    )
    w3p = np.ascontiguousarray(w3p)
    b12p = np.ascontiguousarray(
        b12.astype(np.float32).reshape(JT, P).T)
    b3p = np.ascontiguousarray(
        b3.astype(np.float32).reshape(MT, P).T)

    in_maps = []
    for c in range(NCORES):
        xc = xf[c * T:(c + 1) * T]    # [T, K]
        xp = (
            xc.astype(BF16).T          # [K, T]
            .reshape(KT, P, T)
            .transpose(1, 0, 2)        # [p, kt, t]
            .reshape(P, KT * T)
        )
        in_maps.append({
            "xp": np.ascontiguousarray(xp),
            "w12p": w12p,
            "w3p": w3p,
            "b12p": b12p,
            "b3p": b3p,
        })
    return in_maps


def kernel(x, w12, b12, w3, b3):
    global LAST_EXEC_NS
    from concourse.bass_utils import run_bass_kernel_spmd

    nc = _get_compiled()
    in_maps = _pack_inputs(x, w12, b12, w3, b3)
    trace = os.environ.get("KERNEL_TRACE", "0") == "1"
    res = run_bass_kernel_spmd(
        nc, in_maps, core_ids=list(range(NCORES)), trace=trace
    )
    LAST_EXEC_NS = res.exec_time_ns

    outs = []
    for c in range(NCORES):
        o = res.results[c]["out"]               # [MT, P, T]
        outs.append(np.transpose(o, (2, 0, 1)).reshape(T, M))
    y = np.concatenate(outs, axis=0).reshape(B, S, M)
    return np.ascontiguousarray(y, dtype=np.float32)
